# revision 5
# baseline (speedup 1.0000x reference)
"""AttnBlock3D Trainium2 kernel (8-core frame-parallel).

Math (per reference):
  hn = GroupNorm32(x) * gamma + beta          # stats over (c/32, t, h, w) -> global over frames
  q/k/v = hn @ w{q,k,v} + b{q,k,v}            # per-frame, per-position linear over channels
  attn  = softmax(q @ k.T / sqrt(c))          # per frame, positions hw=4096
  o     = attn @ v @ wp + bp
  out   = x + o

Distribution: one frame (b*t = 8) per NeuronCore. GroupNorm stats need a
cross-frame reduction: each core computes per-channel sum/sumsq over its
frame, a 4KB AllReduce combines them, then everything else is local.

On-chip layouts (SBUF partitions x free):
  XN  [c=512 (4x128), pos=4096] bf16     normalized activations, transposed
  KT  [c_out (4x128), pos=4096] bf16     k transposed
  V   [pos (32x128), c_out=512] bf16     v natural
  per q-block (512 positions):
    QT [c_out (4x128), 512] bf16
    S  psum [128 k-pos, 512 q]  -> exp -> P [k-pos (32x128), 512 q] bf16
    d  psum [1, 512] = ones.T @ P  (softmax denominators via PE)
    O  psum [128 c, 512 q] = V.T @ P; normalized by 1/d (gpsimd partition
       broadcast), projected by wp, + bp' + x (f32 residual), DMA out.
  Biases: q,k as per-partition ACT bias; v+p folded: bp' = wp.T @ bv + bp.
"""

import sys

sys.path.insert(0, "/opt/trn_rl_repo")

import numpy as np

import concourse.bacc as bacc
import concourse.bass as bass
import concourse.mybir as mybir
import concourse.tile as tile
from concourse.bass_utils import run_bass_kernel_spmd

N_CORES = 8
C = 512  # channels
S = 4096  # positions per frame (h*w)
G = 32  # groups
CPG = C // G  # 16 channels per group
PCH = C // 128  # 4 channel chunks of 128 partitions
KCH = S // 128  # 32 position chunks of 128
QB = 512  # q-block size
NQB = S // QB  # 8 q blocks
NB = S // QB
NTOT = CPG * 8 * S  # group-norm element count per group (global over 8 frames)
EPS = 1e-6
SCALE = float(C) ** -0.5

F32 = mybir.dt.float32
BF16 = mybir.dt.bfloat16
AF = mybir.ActivationFunctionType
ALU = mybir.AluOpType
AX = mybir.AxisListType

_NC_CACHE = {}


def build_nc():
    nc = bacc.Bacc("TRN2", target_bir_lowering=False, debug=False, num_devices=N_CORES)

    x_in = nc.dram_tensor("x", [C, S], F32, kind="ExternalInput")
    gamma_in = nc.dram_tensor("gamma", [C], F32, kind="ExternalInput")
    beta_in = nc.dram_tensor("beta", [C], F32, kind="ExternalInput")
    w_in = {}
    b_in = {}
    for nm in ("wq", "wk", "wv", "wp"):
        w_in[nm] = nc.dram_tensor(nm, [C, C], F32, kind="ExternalInput")
    for nm in ("bq", "bk", "bv", "bp"):
        b_in[nm] = nc.dram_tensor(nm, [C], F32, kind="ExternalInput")
    out_d = nc.dram_tensor("out", [C, S], F32, kind="ExternalOutput")

    with tile.TileContext(nc) as tc:
        with (
            # persistent tensors for the whole kernel
            tc.tile_pool(name="persist", bufs=1) as pp,
            tc.tile_pool(name="psum", bufs=1, space="PSUM") as psp,
            tc.tile_pool(name="dram", bufs=1, space="DRAM") as dram,
        ):
            pl = None  # prologue pool, set below
            # ---- persistent SBUF ----
            XN = [pp.tile([128, S], BF16, name=f"XN{p}") for p in range(PCH)]
            KT = [pp.tile([128, S], BF16, name=f"KT{m}") for m in range(PCH)]
            V = [pp.tile([128, C], BF16, name=f"V{j}") for j in range(KCH)]
            W = {
                nm: [pp.tile([128, C], BF16, name=f"{nm}_{p}") for p in range(PCH)]
                for nm in ("wq", "wk", "wv", "wp")
            }
            bq_p = [pp.tile([128, 1], F32, name=f"bqp{p}") for p in range(PCH)]
            bk_p = [pp.tile([128, 1], F32, name=f"bkp{p}") for p in range(PCH)]
            bv_bf = [pp.tile([128, 1], BF16, name=f"bvb{p}") for p in range(PCH)]
            bpp_p = [pp.tile([128, 1], F32, name=f"bppp{p}") for p in range(PCH)]
            sc_p = [pp.tile([128, 1], F32, name=f"scp{p}") for p in range(PCH)]
            bc_p = [pp.tile([128, 1], F32, name=f"bcp{p}") for p in range(PCH)]
            ones_bf = pp.tile([128, 1], BF16, name="ones_bf")
            nc.vector.memset(ones_bf[:], 1.0)

            # ---- prologue pool (released before attention main loop) ----
            prolog_cm = tc.tile_pool(name="prolog", bufs=1)
            pl = prolog_cm.__enter__()

            # ---- weight load + cast (streamed) ----
            for nm in ("wq", "wk", "wv", "wp"):
                for p in range(PCH):
                    wstg = pl.tile([128, C], F32, name="wstg", tag="wstg", bufs=2)
                    nc.sync.dma_start(wstg[:], w_in[nm][p * 128 : (p + 1) * 128, :])
                    nc.vector.tensor_copy(W[nm][p][:], wstg[:])

            # bias loads: partition layout [128,1] x4 each
            for p in range(PCH):
                nc.sync.dma_start(bq_p[p][:], b_in["bq"][p * 128 : (p + 1) * 128, None])
                nc.sync.dma_start(bk_p[p][:], b_in["bk"][p * 128 : (p + 1) * 128, None])
            bv_st = [pl.tile([128, 1], F32, name=f"bvst{p}") for p in range(PCH)]
            for p in range(PCH):
                nc.sync.dma_start(bv_st[p][:], b_in["bv"][p * 128 : (p + 1) * 128, None])
                nc.vector.tensor_copy(bv_bf[p][:], bv_st[p][:])

            # free-layout vectors on partition 0
            gam_sb = pl.tile([1, C], F32, name="gam_sb")
            bet_sb = pl.tile([1, C], F32, name="bet_sb")
            bp_sb = pl.tile([1, C], F32, name="bp_sb")
            nc.sync.dma_start(gam_sb[:], gamma_in[None, :])
            nc.sync.dma_start(bet_sb[:], beta_in[None, :])
            nc.sync.dma_start(bp_sb[:], b_in["bp"][None, :])

            # ---- bp' = wp.T @ bv + bp (fold v-bias + p-bias) ----
            ps_d = psp.tile([1, QB], F32, name="ps_bp", tag="ps_d")
            for ci in range(PCH):
                nc.tensor.matmul(
                    ps_d[:],
                    bv_bf[ci][:],
                    W["wp"][ci][:],
                    start=(ci == 0),
                    stop=(ci == PCH - 1),
                )
            bpp_sb = pl.tile([1, C], F32, name="bpp_sb")
            nc.vector.tensor_tensor(bpp_sb[:], ps_d[:], bp_sb[:], op=ALU.add)

            # ---- pass 1: stream x, per-channel sum & sumsq ----
            sum_t = [pl.tile([128, 1], F32, name=f"sum{p}") for p in range(PCH)]
            ssq_t = [pl.tile([128, 1], F32, name=f"ssq{p}") for p in range(PCH)]
            for p in range(PCH):
                xs = pl.tile([128, S], F32, name="xs", tag="xstream", bufs=2)
                nc.sync.dma_start(xs[:], x_in[p * 128 : (p + 1) * 128, :])
                nc.vector.reduce_sum(sum_t[p][:], xs[:], axis=AX.X)
                junk = pl.tile([128, S], BF16, name="junk", tag="junk", bufs=1)
                nc.scalar.activation(
                    junk[:], xs[:], AF.Square, accum_out=ssq_t[p][:]
                )

            # pack stats -> [128, 8]: cols 0-3 sums, 4-7 sumsq
            stats8 = pl.tile([128, 8], F32, name="stats8")
            for p in range(PCH):
                nc.vector.tensor_copy(stats8[:, p : p + 1], sum_t[p][:])
                nc.vector.tensor_copy(stats8[:, 4 + p : 5 + p], ssq_t[p][:])

            # ---- AllReduce stats across the 8 cores ----
            cc_in = dram.tile([128, 8], F32, name="cc_in")
            cc_out = dram.tile([128, 8], F32, name="cc_out", addr_space="Shared")
            nc.gpsimd.dma_start(cc_in[:], stats8[:])
            nc.gpsimd.collective_compute(
                "AllReduce",
                ALU.add,
                replica_groups=[list(range(N_CORES))],
                ins=[cc_in.opt()],
                outs=[cc_out.opt()],
            )
            cc_sb = pl.tile([1, 1024], F32, name="cc_sb")
            nc.sync.dma_start(cc_sb[:], cc_out[:].rearrange("p j -> (p j)").unsqueeze(0))

            # ---- group stats -> per-channel scale/bias ----
            # cc_sb flat index i = 128*gm + 8*k + jp  (gm=part//16, k=part%16, jp=col)
            gstats = pl.tile([1, 64], F32, name="gstats")
            nc.vector.tensor_reduce(
                gstats[:].rearrange("a (p gm) -> a p gm", p=8, gm=8),
                cc_sb[:].rearrange("a (gm k p) -> a p gm k", gm=8, k=16, p=8),
                axis=AX.X,
                op=ALU.add,
            )
            gsum = gstats[:, 0:32]
            gsq = gstats[:, 32:64]
            invN = 1.0 / float(NTOT)
            mean = pl.tile([1, 32], F32, name="mean")
            var = pl.tile([1, 32], F32, name="var")
            rstd = pl.tile([1, 32], F32, name="rstd")
            tmp32 = pl.tile([1, 32], F32, name="tmp32")
            nc.vector.tensor_scalar_mul(mean[:], gsum, invN)
            nc.vector.tensor_scalar_mul(var[:], gsq, invN)  # E[x^2]
            nc.vector.tensor_tensor(tmp32[:], mean[:], mean[:], op=ALU.mult)
            nc.vector.tensor_tensor(var[:], var[:], tmp32[:], op=ALU.subtract)
            # rstd = 1/sqrt(var+eps)
            eps_t = pl.tile([1, 1], F32, name="eps_t")
            nc.vector.memset(eps_t[:], EPS)
            nc.scalar.activation(tmp32[:], var[:], AF.Sqrt, bias=eps_t[:])
            nc.vector.reciprocal(rstd[:], tmp32[:])
            # sc = gamma * rstd[g];  bc = beta - mean[g]*sc
            sc_sb = pl.tile([1, C], F32, name="sc_sb")
            bc_sb = pl.tile([1, C], F32, name="bc_sb")
            rstd_b = rstd[:].unsqueeze(2).broadcast_to([1, 32, CPG])
            mean_b = mean[:].unsqueeze(2).broadcast_to([1, 32, CPG])
            gam_v = gam_sb[:].rearrange("a (g c) -> a g c", g=32, c=CPG)
            bet_v = bet_sb[:].rearrange("a (g c) -> a g c", g=32, c=CPG)
            sc_v = sc_sb[:].rearrange("a (g c) -> a g c", g=32, c=CPG)
            bc_v = bc_sb[:].rearrange("a (g c) -> a g c", g=32, c=CPG)
            nc.vector.tensor_tensor(sc_v, gam_v, rstd_b, op=ALU.mult)
            nc.vector.tensor_tensor(bc_v, mean_b, sc_v, op=ALU.mult)
            nc.vector.tensor_tensor(bc_v, bet_v, bc_v, op=ALU.subtract)

            # ---- roundtrip free->partition layout via DRAM ----
            scr = dram.tile([3, C], F32, name="scr")
            nc.sync.dma_start(scr[0, None, :], sc_sb[:])
            nc.sync.dma_start(scr[1, None, :], bc_sb[:])
            nc.sync.dma_start(scr[2, None, :], bpp_sb[:])
            for p in range(PCH):
                nc.sync.dma_start(sc_p[p][:], scr[0, p * 128 : (p + 1) * 128, None])
                nc.sync.dma_start(bc_p[p][:], scr[1, p * 128 : (p + 1) * 128, None])
                nc.sync.dma_start(bpp_p[p][:], scr[2, p * 128 : (p + 1) * 128, None])

            # ---- pass 2: re-stream x, normalize -> XN bf16 ----
            for p in range(PCH):
                xs2 = pl.tile([128, S], F32, name="xs2", tag="xstream", bufs=2)
                nc.sync.dma_start(xs2[:], x_in[p * 128 : (p + 1) * 128, :])
                nc.vector.tensor_scalar(
                    XN[p][:],
                    xs2[:],
                    sc_p[p][:],
                    bc_p[p][:],
                    op0=ALU.mult,
                    op1=ALU.add,
                )

            prolog_cm.__exit__(None, None, None)

            # ---- main-loop pool ----
            mainloop_cm = tc.tile_pool(name="mainloop", bufs=1)
            ml = mainloop_cm.__enter__()

            # ---- K^T and V ----
            for m in range(PCH):
                for n in range(NB):
                    ps_k = psp.tile([128, QB], F32, name="ps_k", tag="ps_s", bufs=2)
                    for ci in range(PCH):
                        nc.tensor.matmul(
                            ps_k[:],
                            W["wk"][ci][:, m * 128 : (m + 1) * 128],
                            XN[ci][:, n * QB : (n + 1) * QB],
                            start=(ci == 0),
                            stop=(ci == PCH - 1),
                        )
                    nc.scalar.activation(
                        KT[m][:, n * QB : (n + 1) * QB],
                        ps_k[:],
                        AF.Identity,
                        bias=bk_p[m][:],
                    )
            for j in range(KCH):
                ps_v = psp.tile([128, C], F32, name="ps_v", tag="ps_s", bufs=2)
                for ci in range(PCH):
                    nc.tensor.matmul(
                        ps_v[:],
                        XN[ci][:, j * 128 : (j + 1) * 128],
                        W["wv"][ci][:],
                        start=(ci == 0),
                        stop=(ci == PCH - 1),
                    )
                nc.vector.tensor_copy(V[j][:], ps_v[:])

            # ---- attention main loop over q-blocks ----
            for qb in range(NQB):
                q0 = qb * QB
                # q^T for this block (+bias via ACT)
                QT = [
                    ml.tile([128, QB], BF16, name=f"QT{m}", tag=f"QT{m}", bufs=2)
                    for m in range(PCH)
                ]
                for m in range(PCH):
                    ps_q = psp.tile([128, QB], F32, name="ps_q", tag="ps_q", bufs=1)
                    for ci in range(PCH):
                        nc.tensor.matmul(
                            ps_q[:],
                            W["wq"][ci][:, m * 128 : (m + 1) * 128],
                            XN[ci][:, q0 : q0 + QB],
                            start=(ci == 0),
                            stop=(ci == PCH - 1),
                        )
                    nc.scalar.activation(
                        QT[m][:], ps_q[:], AF.Identity, bias=bq_p[m][:]
                    )

                # scores + exp: S^T[j-chunk] = K^T_j.T @ Q^T -> exp -> P[j]
                P = [
                    ml.tile([128, QB], BF16, name=f"P{j}", tag=f"P{j}", bufs=1)
                    for j in range(KCH)
                ]
                for j in range(KCH):
                    ps_s = psp.tile([128, QB], F32, name="ps_s", tag="ps_s", bufs=2)
                    for ci in range(PCH):
                        nc.tensor.matmul(
                            ps_s[:],
                            KT[ci][:, j * 128 : (j + 1) * 128],
                            QT[ci][:],
                            start=(ci == 0),
                            stop=(ci == PCH - 1),
                        )
                    nc.scalar.activation(P[j][:], ps_s[:], AF.Exp, scale=SCALE)

                # denominators d = ones.T @ P (accumulated over j) and
                # O^T[c] = sum_j V[j].T @ P[j]
                ps_dd = psp.tile([1, QB], F32, name="ps_dd", tag="ps_d", bufs=1)
                ps_o = [
                    psp.tile([128, QB], F32, name=f"ps_o{mc}", tag=f"ps_o{mc}", bufs=1)
                    for mc in range(PCH)
                ]
                for j in range(KCH):
                    nc.tensor.matmul(
                        ps_dd[:],
                        ones_bf[:],
                        P[j][:],
                        start=(j == 0),
                        stop=(j == KCH - 1),
                    )
                    for mc in range(PCH):
                        nc.tensor.matmul(
                            ps_o[mc][:],
                            V[j][:, mc * 128 : (mc + 1) * 128],
                            P[j][:],
                            start=(j == 0),
                            stop=(j == KCH - 1),
                        )

                # r = 1/d broadcast over partitions
                r_sb = ml.tile([1, QB], F32, name="r_sb", tag="r_sb", bufs=2)
                r_bc = ml.tile([128, QB], F32, name="r_bc", tag="r_bc", bufs=2)
                nc.vector.reciprocal(r_sb[:], ps_dd[:])
                nc.gpsimd.partition_broadcast(r_bc[:], r_sb[:])

                # normalize -> bf16
                OT = [
                    ml.tile([128, QB], BF16, name=f"OT{mc}", tag=f"OT{mc}", bufs=1)
                    for mc in range(PCH)
                ]
                for mc in range(PCH):
                    nc.vector.tensor_tensor(
                        OT[mc][:], ps_o[mc][:], r_bc[:], op=ALU.mult
                    )

                # project by wp, add bp' and residual x, write out
                for m in range(PCH):
                    ps_f = psp.tile([128, QB], F32, name="ps_f", tag="ps_q", bufs=1)
                    for mc in range(PCH):
                        nc.tensor.matmul(
                            ps_f[:],
                            W["wp"][mc][:, m * 128 : (m + 1) * 128],
                            OT[mc][:],
                            start=(mc == 0),
                            stop=(mc == PCH - 1),
                        )
                    xr = ml.tile([128, QB], F32, name="xr", tag="xr", bufs=4)
                    nc.sync.dma_start(
                        xr[:], x_in[m * 128 : (m + 1) * 128, q0 : q0 + QB]
                    )
                    os_ = ml.tile([128, QB], F32, name="os", tag="os", bufs=4)
                    nc.vector.scalar_tensor_tensor(
                        os_[:],
                        ps_f[:],
                        bpp_p[m][:],
                        xr[:],
                        op0=ALU.add,
                        op1=ALU.add,
                    )
                    nc.sync.dma_start(
                        out_d[m * 128 : (m + 1) * 128, q0 : q0 + QB], os_[:]
                    )

            mainloop_cm.__exit__(None, None, None)

    nc.compile()
    return nc


def _get_nc():
    if "nc" not in _NC_CACHE:
        _NC_CACHE["nc"] = build_nc()
    return _NC_CACHE["nc"]


def kernel(x, gamma, beta, wq, bq, wk, bk, wv, bv, wp, bp, **_unused):
    x = np.asarray(x, np.float32)
    b, c, t, h, w = x.shape
    assert (b, c, t, h, w) == (1, C, 8, 64, 64)
    nc = _get_nc()

    shared = {
        "gamma": np.ascontiguousarray(np.asarray(gamma, np.float32)),
        "beta": np.ascontiguousarray(np.asarray(beta, np.float32)),
        "wq": np.ascontiguousarray(np.asarray(wq, np.float32)),
        "bq": np.ascontiguousarray(np.asarray(bq, np.float32)),
        "wk": np.ascontiguousarray(np.asarray(wk, np.float32)),
        "bk": np.ascontiguousarray(np.asarray(bk, np.float32)),
        "wv": np.ascontiguousarray(np.asarray(wv, np.float32)),
        "bv": np.ascontiguousarray(np.asarray(bv, np.float32)),
        "wp": np.ascontiguousarray(np.asarray(wp, np.float32)),
        "bp": np.ascontiguousarray(np.asarray(bp, np.float32)),
    }
    in_maps = []
    for ti in range(t):
        frame = np.ascontiguousarray(x[0, :, ti, :, :].reshape(C, S))
        in_maps.append({"x": frame, **shared})

    res = run_bass_kernel_spmd(nc, in_maps, core_ids=list(range(N_CORES)))

    out = np.empty((1, C, t, h, w), np.float32)
    for ti in range(t):
        out[0, :, ti, :, :] = res.results[ti]["out"].reshape(C, h, w)
    return out


# revision 11
# speedup vs baseline: 1.1167x; 1.1167x over previous
"""AttnBlock3D Trainium2 kernel (8-core frame-parallel).

Math (per reference):
  hn = GroupNorm32(x) * gamma + beta          # stats over (c/32, t, h, w) -> global over frames
  q/k/v = hn @ w{q,k,v} + b{q,k,v}            # per-frame, per-position linear over channels
  attn  = softmax(q @ k.T / sqrt(c))          # per frame, positions hw=4096
  o     = attn @ v @ wp + bp
  out   = x + o

Distribution: one frame (b*t = 8) per NeuronCore. GroupNorm stats need a
cross-frame reduction: each core computes per-channel sum/sumsq over its
frame, a 4KB AllReduce combines them, then everything else is local.
Group-stat math (16-channel segment sums, group->channel broadcast) runs on
the PE via tiny indicator-matrix matmuls to avoid slow 1-partition DVE ops.

On-chip layouts (SBUF partitions x free):
  XN  [c=512 (4x128), pos=4096] bf16     normalized activations, transposed
  KT  [c_out (4x128), pos=4096] bf16     k transposed
  V   [pos (32x128), c_out=512] bf16     v natural
  per q-block (512 positions), flash-pipelined over k-chunks j:
    S_j psum [128, 512] -> exp -> P_j bf16 (few rotating slots)
    d  psum [1,512] += ones.T @ P_j   (softmax denominators on PE)
    O  psum [128c, 512] += V_j.T @ P_j
    r = recip(bcast(d)); OT = O * r; out = wp.T @ OT + bp' + x (f32 residual)
  Biases: q,k as per-partition ACT bias; v,p folded: bp' = wp.T @ bv + bp.
"""

import sys

sys.path.insert(0, "/opt/trn_rl_repo")

import numpy as np

import concourse.bacc as bacc
import concourse.bass as bass
import concourse.mybir as mybir
import concourse.tile as tile
from concourse.bass_utils import run_bass_kernel_spmd

N_CORES = 8
C = 512  # channels
S = 4096  # positions per frame (h*w)
G = 32  # groups
CPG = C // G  # 16 channels per group
PCH = C // 128  # 4 channel chunks of 128 partitions
KCH = S // 128  # 32 position chunks of 128
QB = 512  # q-block size
NQB = S // QB  # 8 q blocks
NTOT = CPG * 8 * S  # group-norm element count per group (global over 8 frames)
EPS = 1e-6
SCALE = float(C) ** -0.5

F32 = mybir.dt.float32
BF16 = mybir.dt.bfloat16
AF = mybir.ActivationFunctionType
ALU = mybir.AluOpType
AX = mybir.AxisListType

_NC_CACHE = {}
DEBUG = False


def build_nc():
    nc = bacc.Bacc("TRN2", target_bir_lowering=False, debug=False, num_devices=N_CORES)

    x_in = nc.dram_tensor("x", [C, S], F32, kind="ExternalInput")
    gamma_in = nc.dram_tensor("gamma", [C], F32, kind="ExternalInput")
    beta_in = nc.dram_tensor("beta", [C], F32, kind="ExternalInput")
    w_in = {}
    b_in = {}
    for nm in ("wq", "wk", "wv", "wp"):
        w_in[nm] = nc.dram_tensor(nm, [C, C], F32, kind="ExternalInput")
    for nm in ("bq", "bk", "bv", "bp"):
        b_in[nm] = nc.dram_tensor(nm, [C], F32, kind="ExternalInput")
    out_d = nc.dram_tensor("out", [C, S], F32, kind="ExternalOutput")
    dbg_d = nc.dram_tensor("dbg", [128, 64], F32, kind="ExternalOutput") if DEBUG else None

    with tile.TileContext(nc) as tc:
        with (
            tc.tile_pool(name="persist", bufs=1) as pp,
            tc.tile_pool(name="psum", bufs=1, space="PSUM") as psp,
            tc.tile_pool(name="dram", bufs=1, space="DRAM") as dram,
        ):
            # ---- persistent SBUF ----
            XN = [pp.tile([128, S], BF16, name=f"XN{p}") for p in range(PCH)]
            KT = [pp.tile([128, S], BF16, name=f"KT{m}") for m in range(PCH)]
            V = [pp.tile([128, C], BF16, name=f"V{j}") for j in range(KCH)]
            W = {
                nm: [pp.tile([128, C], BF16, name=f"{nm}_{p}") for p in range(PCH)]
                for nm in ("wq", "wk", "wv", "wp")
            }
            bq_p = [pp.tile([128, 1], F32, name=f"bqp{p}") for p in range(PCH)]
            bk_p = [pp.tile([128, 1], F32, name=f"bkp{p}") for p in range(PCH)]
            bv_bf = [pp.tile([128, 1], BF16, name=f"bvb{p}") for p in range(PCH)]
            bpp_p = [pp.tile([128, 1], F32, name=f"bppp{p}") for p in range(PCH)]
            sc_p = [pp.tile([128, 1], F32, name=f"scp{p}") for p in range(PCH)]
            bc_p = [pp.tile([128, 1], F32, name=f"bcp{p}") for p in range(PCH)]
            ones_bf = pp.tile([128, 1], BF16, name="ones_bf")
            nc.vector.memset(ones_bf[:], 1.0)

            # gpsimd ucode warmup (first custom op pays a library load)
            gw_in = pp.tile([1, 8], F32, name="gw_in")
            gw_out = pp.tile([128, 8], F32, name="gw_out")
            nc.vector.memset(gw_in[:], 0.0)
            nc.gpsimd.partition_broadcast(gw_out[:], gw_in[:])

            # ---- prologue pool (released before attention main loop) ----
            prolog_cm = tc.tile_pool(name="prolog", bufs=1)
            pl = prolog_cm.__enter__()

            # ---- pass 1 first: stream x (critical path), sum & sumsq ----
            sum_t = [pl.tile([128, 1], F32, name=f"sum{p}") for p in range(PCH)]
            ssq_t = [pl.tile([128, 1], F32, name=f"ssq{p}") for p in range(PCH)]
            for p in range(PCH):
                xs = pl.tile([128, S], F32, name="xs", tag="xstream", bufs=2)
                nc.sync.dma_start(xs[:], x_in[p * 128 : (p + 1) * 128, :])
                nc.vector.reduce_sum(sum_t[p][:], xs[:], axis=AX.X)
                junk = pl.tile([128, S], BF16, name="junk", tag="junk", bufs=1)
                nc.scalar.activation(junk[:], xs[:], AF.Square, accum_out=ssq_t[p][:])

            # pack stats -> [128, 8]: cols 0-3 sums, 4-7 sumsq; AllReduce
            stats8 = pl.tile([128, 8], F32, name="stats8")
            for p in range(PCH):
                nc.vector.tensor_copy(stats8[:, p : p + 1], sum_t[p][:])
                nc.vector.tensor_copy(stats8[:, 4 + p : 5 + p], ssq_t[p][:])
            cc_in = dram.tile([128, 8], F32, name="cc_in")
            cc_out = dram.tile([128, 8], F32, name="cc_out", addr_space="Shared")
            nc.gpsimd.dma_start(cc_in[:], stats8[:])
            nc.gpsimd.collective_compute(
                "AllReduce",
                ALU.add,
                replica_groups=[list(range(N_CORES))],
                ins=[cc_in.opt()],
                outs=[cc_out.opt()],
            )

            # ---- small loads (off critical path) ----
            for p in range(PCH):
                nc.sync.dma_start(bq_p[p][:], b_in["bq"][p * 128 : (p + 1) * 128, None])
                nc.sync.dma_start(bk_p[p][:], b_in["bk"][p * 128 : (p + 1) * 128, None])
            bv_st = [pl.tile([128, 1], F32, name=f"bvst{p}") for p in range(PCH)]
            bp_p = [pl.tile([128, 1], F32, name=f"bpst{p}") for p in range(PCH)]
            gam_p = [pl.tile([128, 1], F32, name=f"gam{p}") for p in range(PCH)]
            bet_p = [pl.tile([128, 1], F32, name=f"bet{p}") for p in range(PCH)]
            for p in range(PCH):
                sl = slice(p * 128, (p + 1) * 128)
                nc.sync.dma_start(bv_st[p][:], b_in["bv"][sl, None])
                nc.sync.dma_start(bp_p[p][:], b_in["bp"][sl, None])
                nc.sync.dma_start(gam_p[p][:], gamma_in[sl, None])
                nc.sync.dma_start(bet_p[p][:], beta_in[sl, None])
                nc.vector.tensor_copy(bv_bf[p][:], bv_st[p][:])

            # ---- weight load + cast (streamed); wp first (bp' needs it) ----
            for nm in ("wp", "wq", "wk", "wv"):
                for p in range(PCH):
                    wstg = pl.tile([128, C], F32, name="wstg", tag="wstg", bufs=2)
                    nc.sync.dma_start(wstg[:], w_in[nm][p * 128 : (p + 1) * 128, :])
                    nc.vector.tensor_copy(W[nm][p][:], wstg[:])

            # indicator matrices for group-segment sums / broadcasts
            ind_np = np.zeros((128, 8), np.float32)  # [part, gl] = part//16==gl
            for gl in range(8):
                ind_np[16 * gl : 16 * (gl + 1), gl] = 1.0
            ind_d = nc.inline_tensor(ind_np, name="ind_const")
            indt_d = nc.inline_tensor(np.ascontiguousarray(ind_np.T), name="indt_const")
            IND = pl.tile([128, 8], F32, name="IND")
            INDT = pl.tile([8, 128], F32, name="INDT")
            nc.sync.dma_start(IND[:], ind_d[:, :])
            nc.sync.dma_start(INDT[:], indt_d[:, :])

            # ---- bp' = wp.T @ bv + bp via N=1 matmuls ----
            for m in range(PCH):
                ps_bp = psp.tile([128, 1], F32, name="ps_bp", tag="ps_d", bufs=1)
                for ci in range(PCH):
                    nc.tensor.matmul(
                        ps_bp[:],
                        W["wp"][ci][:, m * 128 : (m + 1) * 128],
                        bv_bf[ci][:],
                        start=(ci == 0),
                        stop=(ci == PCH - 1),
                    )
                nc.vector.tensor_tensor(bpp_p[m][:], ps_bp[:], bp_p[m][:], op=ALU.add)

            # ---- post-collective: group stats on PE ----
            stats_g = pl.tile([128, 8], F32, name="stats_g")
            nc.sync.dma_start(stats_g[:], cc_out[:])
            ps_g = psp.tile([8, 8], F32, name="ps_g", tag="ps_d", bufs=1)
            # out[gl, j] = sum_part IND[part, gl] * stats_g[part, j]
            nc.tensor.matmul(ps_g[:], IND[:], stats_g[:], start=True, stop=True)
            gs8 = pl.tile([8, 8], F32, name="gs8")
            nc.vector.tensor_copy(gs8[:], ps_g[:])
            # per-group mean/rstd on 8 partitions x 4 chunks
            invN = 1.0 / float(NTOT)
            mean8 = pl.tile([8, 4], F32, name="mean8")
            var8 = pl.tile([8, 4], F32, name="var8")
            rstd8 = pl.tile([8, 4], F32, name="rstd8")
            eps8 = pl.tile([8, 1], F32, name="eps8")
            nc.vector.memset(eps8[:], EPS)
            nc.vector.tensor_scalar_mul(mean8[:], gs8[:, 0:4], invN)
            nc.vector.tensor_scalar_mul(var8[:], gs8[:, 4:8], invN)
            nc.vector.tensor_tensor(rstd8[:], mean8[:], mean8[:], op=ALU.mult)
            nc.vector.tensor_tensor(var8[:], var8[:], rstd8[:], op=ALU.subtract)
            nc.scalar.activation(var8[:], var8[:], AF.Sqrt, bias=eps8[:])
            nc.vector.reciprocal(rstd8[:], var8[:])
            # pack [rstd | mean] and broadcast groups -> 128 partitions via PE
            rm8 = pl.tile([8, 8], F32, name="rm8")
            nc.vector.tensor_copy(rm8[:, 0:4], rstd8[:])
            nc.vector.tensor_copy(rm8[:, 4:8], mean8[:])
            ps_e = psp.tile([128, 8], F32, name="ps_e", tag="ps_d", bufs=1)
            nc.tensor.matmul(ps_e[:], INDT[:], rm8[:], start=True, stop=True)
            # sc = gamma * rstd; bc = beta - mean * sc   (per chunk p)
            for p in range(PCH):
                nc.vector.tensor_tensor(
                    sc_p[p][:], gam_p[p][:], ps_e[:, p : p + 1], op=ALU.mult
                )
                nc.vector.tensor_tensor(
                    bc_p[p][:], ps_e[:, 4 + p : 5 + p], sc_p[p][:], op=ALU.mult
                )
                nc.vector.tensor_tensor(
                    bc_p[p][:], bet_p[p][:], bc_p[p][:], op=ALU.subtract
                )

            if DEBUG:
                dbg_t = pl.tile([128, 64], F32, name="dbg_t")
                nc.vector.memset(dbg_t[:], 0.0)
                nc.vector.tensor_copy(dbg_t[:, 0:8], stats_g[:])
                nc.vector.tensor_copy(dbg_t[0:8, 8:16], gs8[:])
                nc.vector.tensor_copy(dbg_t[0:8, 16:20], mean8[:])
                nc.vector.tensor_copy(dbg_t[0:8, 20:24], var8[:])
                nc.vector.tensor_copy(dbg_t[0:8, 24:28], rstd8[:])
                nc.vector.tensor_copy(dbg_t[:, 28:36], ps_e[:])
                for p in range(PCH):
                    nc.vector.tensor_copy(dbg_t[:, 36 + p : 37 + p], sc_p[p][:])
                    nc.vector.tensor_copy(dbg_t[:, 40 + p : 41 + p], bc_p[p][:])
                    nc.vector.tensor_copy(dbg_t[:, 44 + p : 45 + p], sum_t[p][:])
                    nc.vector.tensor_copy(dbg_t[:, 48 + p : 49 + p], ssq_t[p][:])
                nc.vector.tensor_copy(dbg_t[:, 52:60], IND[:])
                nc.sync.dma_start(dbg_d[:, :], dbg_t[:])

            # ---- pass 2: re-stream x, normalize -> XN bf16 ----
            for p in range(PCH):
                xs2 = pl.tile([128, S], F32, name="xs2", tag="xs2", bufs=2)
                nc.sync.dma_start(xs2[:], x_in[p * 128 : (p + 1) * 128, :])
                nc.vector.tensor_scalar(
                    XN[p][:], xs2[:], sc_p[p][:], bc_p[p][:], op0=ALU.mult, op1=ALU.add
                )

            prolog_cm.__exit__(None, None, None)

            # ---- main-loop pool ----
            mainloop_cm = tc.tile_pool(name="mainloop", bufs=1)
            ml = mainloop_cm.__enter__()

            # ---- K^T (bias via ACT) and V ----
            for m in range(PCH):
                for n in range(NQB):
                    ps_k = psp.tile([128, QB], F32, name="ps_k", tag="ps_s", bufs=2)
                    for ci in range(PCH):
                        nc.tensor.matmul(
                            ps_k[:],
                            W["wk"][ci][:, m * 128 : (m + 1) * 128],
                            XN[ci][:, n * QB : (n + 1) * QB],
                            start=(ci == 0),
                            stop=(ci == PCH - 1),
                        )
                    nc.scalar.activation(
                        KT[m][:, n * QB : (n + 1) * QB],
                        ps_k[:],
                        AF.Identity,
                        bias=bk_p[m][:],
                    )
            for j in range(KCH):
                ps_v = psp.tile([128, C], F32, name="ps_v", tag="ps_s", bufs=2)
                for ci in range(PCH):
                    nc.tensor.matmul(
                        ps_v[:],
                        XN[ci][:, j * 128 : (j + 1) * 128],
                        W["wv"][ci][:],
                        start=(ci == 0),
                        stop=(ci == PCH - 1),
                    )
                nc.vector.tensor_copy(V[j][:], ps_v[:])

            # ---- attention main loop over q-blocks ----
            def emit_qt(qb, m, QT):
                ps_q = psp.tile([128, QB], F32, name="ps_q", tag="ps_q", bufs=1)
                for ci in range(PCH):
                    nc.tensor.matmul(
                        ps_q[:],
                        W["wq"][ci][:, m * 128 : (m + 1) * 128],
                        XN[ci][:, qb * QB : (qb + 1) * QB],
                        start=(ci == 0),
                        stop=(ci == PCH - 1),
                    )
                nc.scalar.activation(QT[m][:], ps_q[:], AF.Identity, bias=bq_p[m][:])

            def make_qt():
                return [
                    ml.tile([128, QB], BF16, name=f"QT{m}", tag=f"QT{m}", bufs=2)
                    for m in range(PCH)
                ]

            QT_cur = make_qt()
            for m in range(PCH):
                emit_qt(0, m, QT_cur)

            def emit_s(j, QT):
                """scores S^T[j] = K^T_j.T @ Q^T -> exp -> P tile (rotating)."""
                ps_s = psp.tile([128, QB], F32, name="ps_s", tag="ps_s", bufs=2)
                for ci in range(PCH):
                    nc.tensor.matmul(
                        ps_s[:],
                        KT[ci][:, j * 128 : (j + 1) * 128],
                        QT[ci][:],
                        start=(ci == 0),
                        stop=(ci == PCH - 1),
                    )
                P = ml.tile([128, QB], BF16, name="P", tag="P", bufs=6)
                nc.scalar.activation(P[:], ps_s[:], AF.Exp, scale=SCALE)
                return P

            for qb in range(NQB):
                QT_next = None
                # software-pipelined: s two ahead, then d_j + PV_j consume P[j]
                ps_dd = psp.tile([1, QB], F32, name="ps_dd", tag="ps_d", bufs=1)
                ps_o = [
                    psp.tile([128, QB], F32, name=f"ps_o{mc}", tag=f"ps_o{mc}", bufs=1)
                    for mc in range(PCH)
                ]
                Ps = [None] * KCH
                Ps[0] = emit_s(0, QT_cur)
                Ps[1] = emit_s(1, QT_cur)
                for j in range(KCH):
                    if j + 2 < KCH:
                        Ps[j + 2] = emit_s(j + 2, QT_cur)
                    nc.tensor.matmul(
                        ps_dd[:],
                        ones_bf[:],
                        Ps[j][:],
                        start=(j == 0),
                        stop=(j == KCH - 1),
                    )
                    for mc in range(PCH):
                        nc.tensor.matmul(
                            ps_o[mc][:],
                            V[j][:, mc * 128 : (mc + 1) * 128],
                            Ps[j][:],
                            start=(j == 0),
                            stop=(j == KCH - 1),
                        )
                    Ps[j] = None
                    # interleave next block's q^T generation into the PV stream
                    if j % 8 == 7 and qb + 1 < NQB:
                        if QT_next is None:
                            QT_next = make_qt()
                        emit_qt(qb + 1, j // 8, QT_next)

                # denominators -> r broadcast (overlaps tail of PV on PE)
                d_sb = ml.tile([1, QB], F32, name="d_sb", tag="d_sb", bufs=2)
                d_bc = ml.tile([128, QB], F32, name="d_bc", tag="d_bc", bufs=2)
                r_bc = ml.tile([128, QB], F32, name="r_bc", tag="r_bc", bufs=2)
                nc.scalar.copy(d_sb[:], ps_dd[:])
                nc.gpsimd.partition_broadcast(d_bc[:], d_sb[:])
                nc.vector.reciprocal(r_bc[:], d_bc[:])

                # normalize -> bf16
                OT = [
                    ml.tile([128, QB], BF16, name=f"OT{mc}", tag=f"OT{mc}", bufs=1)
                    for mc in range(PCH)
                ]
                for mc in range(PCH):
                    nc.vector.tensor_tensor(OT[mc][:], ps_o[mc][:], r_bc[:], op=ALU.mult)

                # project by wp, add bp' and residual x, write out
                q0 = qb * QB
                for m in range(PCH):
                    ps_f = psp.tile([128, QB], F32, name="ps_f", tag="ps_s", bufs=2)
                    for mc in range(PCH):
                        nc.tensor.matmul(
                            ps_f[:],
                            W["wp"][mc][:, m * 128 : (m + 1) * 128],
                            OT[mc][:],
                            start=(mc == 0),
                            stop=(mc == PCH - 1),
                        )
                    xr = ml.tile([128, QB], F32, name="xr", tag="xr", bufs=4)
                    nc.sync.dma_start(xr[:], x_in[m * 128 : (m + 1) * 128, q0 : q0 + QB])
                    os_ = ml.tile([128, QB], F32, name="os", tag="os", bufs=4)
                    nc.vector.scalar_tensor_tensor(
                        os_[:], ps_f[:], bpp_p[m][:], xr[:], op0=ALU.add, op1=ALU.add
                    )
                    nc.sync.dma_start(
                        out_d[m * 128 : (m + 1) * 128, q0 : q0 + QB], os_[:]
                    )
                if QT_next is not None:
                    QT_cur = QT_next

            mainloop_cm.__exit__(None, None, None)

    nc.compile()
    return nc


def _get_nc():
    if "nc" not in _NC_CACHE:
        _NC_CACHE["nc"] = build_nc()
    return _NC_CACHE["nc"]


def kernel(x, gamma, beta, wq, bq, wk, bk, wv, bv, wp, bp, **_unused):
    x = np.asarray(x, np.float32)
    b, c, t, h, w = x.shape
    assert (b, c, t, h, w) == (1, C, 8, 64, 64)
    nc = _get_nc()

    shared = {
        "gamma": np.ascontiguousarray(np.asarray(gamma, np.float32)),
        "beta": np.ascontiguousarray(np.asarray(beta, np.float32)),
        "wq": np.ascontiguousarray(np.asarray(wq, np.float32)),
        "bq": np.ascontiguousarray(np.asarray(bq, np.float32)),
        "wk": np.ascontiguousarray(np.asarray(wk, np.float32)),
        "bk": np.ascontiguousarray(np.asarray(bk, np.float32)),
        "wv": np.ascontiguousarray(np.asarray(wv, np.float32)),
        "bv": np.ascontiguousarray(np.asarray(bv, np.float32)),
        "wp": np.ascontiguousarray(np.asarray(wp, np.float32)),
        "bp": np.ascontiguousarray(np.asarray(bp, np.float32)),
    }
    in_maps = []
    for ti in range(t):
        frame = np.ascontiguousarray(x[0, :, ti, :, :].reshape(C, S))
        in_maps.append({"x": frame, **shared})

    res = run_bass_kernel_spmd(nc, in_maps, core_ids=list(range(N_CORES)))

    out = np.empty((1, C, t, h, w), np.float32)
    for ti in range(t):
        out[0, :, ti, :, :] = res.results[ti]["out"].reshape(C, h, w)
    return out
